# revision 15
# baseline (speedup 1.0000x reference)
"""CenterLoss forward on 8 Trainium2 NeuronCores.

Reference semantics:
    distmat[b, c] = ||x_b||^2 + ||center_c||^2 - 2 <x_b, center_c>
    loss = sum(clip(distmat * onehot(labels), 1e-12, 1e12)) / B

The masked matrix is zero everywhere except (b, labels[b]), and clip() lifts
each of the B*(C-1) zeros to exactly 1e-12.  So:

    loss = ( sum_b clip(||x_b - centers[labels[b]]||^2, 1e-12, 1e12)
             + B*(C-1)*1e-12 ) / B

which needs only a row gather + per-row squared distance, not the full
(B, C) distance matrix (42 GFLOP -> ~4 MFLOP).

Device kernel (raw Bass, single basic block, SPMD data-parallel over batch),
v3 — evolved from v1 (23.6us) and v2 (20.7us) traces:
  - everything in bf16 (x shard, baked centers table): halves DMA bytes and
    doubles DVE throughput; quantization error ~0.2%/element with random
    sign, averages out over the 512-d row sums (tolerance is 2e-2).
  - labels land FIRST via sync-engine HWDGE (625ns issue); the label->gather
    dependency is the critical-path prefix.
  - x shard is pre-transposed on host to [128, NT*D] and loads as ONE
    direct HWDGE DMA.
  - the row gather runs as TWO InstDMAGatherAnt ops (dma_gather, 256 rows
    each) instead of four generic indirect DMACopies: SWDGE cost is ~994ns
    fixed + 0.34ns/descriptor, so batching descriptors into fewer
    instructions wins; two (not one) lets compute on the first half overlap
    the second half's transfer.  Indices are int16 in the uCode's wrapped
    layout: idx k of a gather sits at SBUF [k%16, k//16].
    (v2 lesson: a >2-dim dest AP on a *generic* SWDGE DMACopy falls off the
    fast path — transfers defer to a ~20us poll.  dma_gather natively takes
    the 3D dest AP.)
  - a trailing dummy SWDGE DMA flushes the last gather's completion receipt
    (observed ~2us receipt lag otherwise).
  - compute: DVE does all four bf16 subtracts; squares+row-accumulate are
    split — ACT (Square activation, accum_out) takes subtiles 0,1 while DVE
    (scalar_tensor_tensor) takes 2,3.  The ACT Square table is primed at
    program start so its 1.28us load hides under the DMAs.
  - no device clip: the host applies np.clip before summing (it already
    owns the final reduction); saves the DVE clip + a sem hop on the tail.
  - output [128, 4] f32 row distances via sync HWDGE.
  - sync rules (inherited): every DMA completion that matters gets its own
    sem; SWDGE sems never shared with HWDGE; same-engine RAW on DVE via
    dve_sem counts; cross-engine edges via explicit waits.
  - host: sum clipped distances in f64, add the analytic clip floor
    B*(C-1)*1e-12, divide by B.
"""

import hashlib
from contextlib import ExitStack

import ml_dtypes
import numpy as np

import concourse.bass as bass
from concourse import library_config, library_overlay, mybir
from concourse.bass_utils import run_bass_kernel_spmd

B = 4096
D = 512
C = 10000
NCORES = 8
BL = B // NCORES          # 512 rows per core
P = 128                   # partitions
NT = BL // P              # 4 subtiles of 128 rows per core
NG = 2                    # dma_gather ops per core
RPG = BL // NG            # rows per gather (256)
ICOLS = RPG // 16         # idx columns per gather in the wrapped layout (16)

F32 = mybir.dt.float32
BF16 = mybir.dt.bfloat16
I16 = mybir.dt.int16

_CACHE = {}


def legalize_waits(nc, max_waits=1):
    """The walrus build in this container accepts at most one embedded
    sem-wait per TPB instruction ("Too many sync wait commands" otherwise).
    Split any excess into standalone single-wait InstEventSemaphore no-ops
    immediately before the instruction on the same engine — engine program
    order then enforces the identical synchronization."""
    n_split = 0
    for f in nc.m.functions:
        for b in f.blocks:
            insts = list(b.instructions)
            out = []
            for inst in insts:
                si = inst.sync_info
                waits = list(si.on_wait) if (si is not None and si.on_wait) else []
                if len(waits) > max_waits:
                    keep = waits[-max_waits:]
                    spill = waits[:-max_waits]
                    for k, w in enumerate(spill):
                        out.append(
                            mybir.InstEventSemaphore(
                                name=f"{inst.name}-lw{k}",
                                engine=inst.engine,
                                sync_info=mybir.SyncInfo(on_wait=[w], on_update=[]),
                            )
                        )
                        n_split += 1
                    inst.sync_info = mybir.SyncInfo(
                        on_wait=keep, on_update=list(si.on_update or [])
                    )
                out.append(inst)
            b.instructions = out
    return n_split


def build_nc(centers_np):
    nc = bass.Bass(num_swdge_queues=2)

    # host pre-arranges x: [p, t*D + d] = x_core[t*128 + p, d], bf16
    x = nc.dram_tensor("x", [P, NT * D], BF16, kind="ExternalInput")
    # labels in the dma_gather wrapped layout (int16):
    #   gather g's idx k  ->  [k % 16, g*ICOLS + k // 16]
    labels = nc.dram_tensor("labels", [P, NG * ICOLS], I16, kind="ExternalInput")
    out = nc.dram_tensor("out", [P, NT], F32, kind="ExternalOutput")
    centers = nc.inline_tensor(
        np.ascontiguousarray(centers_np.astype(ml_dtypes.bfloat16)), name="centers"
    )

    es = ExitStack()
    idx_sb = es.enter_context(nc.sbuf_tensor("idx_sb", [P, NG * ICOLS], I16))
    x_sb = es.enter_context(nc.sbuf_tensor("x_sb", [P, NT * D], BF16))
    c_sb = es.enter_context(nc.sbuf_tensor("c_sb", [P, NT * D], BF16))
    df_sb = es.enter_context(nc.sbuf_tensor("df_sb", [P, NT * D], BF16))
    sq_act = es.enter_context(nc.sbuf_tensor("sq_act", [P, D], BF16))
    sq_dve = es.enter_context(nc.sbuf_tensor("sq_dve", [P, D], BF16))
    prime_sb = es.enter_context(nc.sbuf_tensor("prime_sb", [P, 1], BF16))
    dcol = es.enter_context(nc.sbuf_tensor("dcol", [P, NT], F32))
    scr_sb = es.enter_context(nc.sbuf_tensor("scr_sb", [P, 8], I16))

    lbl_sem = es.enter_context(nc.semaphore("lbl_sem"))
    x_sem = es.enter_context(nc.semaphore("x_sem"))
    g_sems = [es.enter_context(nc.semaphore(f"g_sem{g}")) for g in range(NG)]
    f_sem = es.enter_context(nc.semaphore("f_sem"))
    dve_sem = es.enter_context(nc.semaphore("dve_sem"))
    act_sem = es.enter_context(nc.semaphore("act_sem"))
    o_sem = es.enter_context(nc.semaphore("o_sem"))

    # ---- sync/HWDGE: labels first (critical-path prefix), then x ----
    nc.sync.dma_start(out=idx_sb[:, :], in_=labels[:, :]).then_inc(lbl_sem, 16)
    nc.sync.dma_start(out=x_sb[:, :], in_=x[:, :]).then_inc(x_sem, 16)

    # ---- scalar/ACT: prime the Square activation table under the DMAs ----
    nc.scalar.activation(
        out=prime_sb[:, :], in_=prime_sb[:, :],
        func=mybir.ActivationFunctionType.Square,
    )

    # ---- gpsimd: batched row gathers as soon as the indices land ----
    # InstDMAGatherAnt lives in the mlp uCode library; reload the Q7 with it
    # immediately (hides under the label DMA flight)
    nc.gpsimd.load_library(library_config.mlp)
    nidx_reg = nc.gpsimd.to_reg(RPG)
    nc.gpsimd.wait_ge(lbl_sem, 16)
    for g in range(NG):
        nc.gpsimd.dma_gather(
            out_ap=c_sb[:, g * RPG * D // P:(g + 1) * RPG * D // P].rearrange(
                "p (t e) -> p t e", e=D
            ),
            in_ap=centers[:],
            idxs_ap=idx_sb[:, g * ICOLS:(g + 1) * ICOLS],
            num_idxs=RPG,
            num_idxs_reg=nidx_reg,
            elem_size=D,
            queue_num=g % 2,
        ).then_inc(g_sems[g], 16)
    # trailing dummy SWDGE DMA: flushes the last gather's completion receipt
    nc.gpsimd.dma_start(out=scr_sb[:, :], in_=labels[:, 0:8]).then_inc(f_sem, 16)

    # ---- DVE: bf16 subtracts (all four); squares split DVE/ACT ----
    nc.vector.wait_ge(x_sem, 16)
    n_dve = 0
    for t in range(NT):
        cs = slice(t * D, (t + 1) * D)
        nc.vector.wait_ge(g_sems[t * NG // NT], 16)
        nc.vector.tensor_tensor(
            out=df_sb[:, cs],
            in0=x_sb[:, cs],
            in1=c_sb[:, cs],
            op=mybir.AluOpType.subtract,
        ).then_inc(dve_sem, 1)
        n_dve += 1
    # ACT squares subtiles 0,1 (each waits its subtract)
    for t in range(2):
        cs = slice(t * D, (t + 1) * D)
        nc.scalar.wait_ge(dve_sem, t + 1)
        nc.scalar.activation(
            out=sq_act[:, :],
            in_=df_sb[:, cs],
            func=mybir.ActivationFunctionType.Square,
            accum_out=dcol[:, t:t + 1],
        ).then_inc(act_sem, 1)
    # DVE squares subtiles 2,3 (same-engine RAW on DVE still needs an
    # explicit sem edge — v1 lesson from the race detector + hardware)
    for t in range(2, NT):
        cs = slice(t * D, (t + 1) * D)
        nc.vector.wait_ge(dve_sem, t + 1)
        nc.vector.scalar_tensor_tensor(
            out=sq_dve[:, :],
            in0=df_sb[:, cs],
            scalar=1.0,
            in1=df_sb[:, cs],
            op0=mybir.AluOpType.mult,
            op1=mybir.AluOpType.mult,
            accum_out=dcol[:, t:t + 1],
        ).then_inc(dve_sem, 1)
        n_dve += 1

    # ---- result out via sync HWDGE; runtime drains rings before reading ----
    nc.sync.wait_ge(act_sem, 2)
    nc.sync.wait_ge(dve_sem, n_dve)
    nc.sync.dma_start(out=out[:, :], in_=dcol[:, :]).then_inc(o_sem, 16)

    # NOTE: the ExitStack is intentionally NOT closed — closing would free
    # the semaphores and emit an expensive end-of-program drain + barrier;
    # Bass already clears the whole sem range in its preamble, so repeated
    # executions stay safe without it.
    legalize_waits(nc)
    # raw Bass skips Bacc's codegen_inst_isa_subclasses pass; without it the
    # PseudoReloadLibraryIndex lowers with empty .instr -> "ISA wrong length"
    library_overlay.lower_extended_insts(nc)
    return nc


def _get_nc(centers_np):
    arr = np.ascontiguousarray(centers_np, np.float32)
    key = hashlib.md5(arr.tobytes()).hexdigest()
    if _CACHE.get("key") != key:
        _CACHE["nc"] = build_nc(arr)
        _CACHE["key"] = key
    return _CACHE["nc"]


def make_in_maps(x, labels, centers=None):
    xb = np.asarray(x, dtype=np.float32).astype(ml_dtypes.bfloat16)
    # [p, t*D + d] = x[core*512 + t*128 + p, d]
    xb = np.ascontiguousarray(
        xb.reshape(NCORES, NT, P, D).transpose(0, 2, 1, 3).reshape(NCORES, P, NT * D)
    )
    # wrapped int16 idx layout: gather g's idx k -> [k%16, g*ICOLS + k//16]
    lab = np.asarray(labels).astype(np.int16).reshape(NCORES, NG, ICOLS, 16)
    lab_w = np.zeros((NCORES, P, NG * ICOLS), dtype=np.int16)
    lab_w[:, :16, :] = lab.transpose(0, 3, 1, 2).reshape(NCORES, 16, NG * ICOLS)
    return [{"x": xb[i], "labels": lab_w[i]} for i in range(NCORES)]


def finalize(results):
    total = 0.0
    for r in results:
        v = np.asarray(r["out"], dtype=np.float64)
        total += float(np.clip(v, 1e-12, 1e12).sum())
    loss = (total + B * (C - 1) * 1e-12) / B
    return np.array(loss, dtype=np.float32)


def kernel(x, labels, centers):
    nc = _get_nc(centers)
    in_maps = make_in_maps(x, labels)
    res = run_bass_kernel_spmd(nc, in_maps, core_ids=list(range(NCORES)))
    return finalize(res.results)
